# revision 1
# baseline (speedup 1.0000x reference)
"""Trainium2 Bass kernel for modulated (StyleGAN2-style) 3x3 conv, groups=batch.

Full-input contract: kernel(**inputs) takes the unsharded numpy inputs and
returns the full (16, 512, 64, 64) fp32 output. Internally the batch is
sharded 2-per-core across 8 NeuronCores (data parallel); weight/mod params
are replicated.

Math (matching the jax reference):
    s      = style @ mod_w.T + mod_b                      # (B, IC)
    wmod   = SCALE * w * s[:,None,:,None,None]
    demod  = rsqrt(sum(wmod^2, (ic,kh,kw)) + 1e-8)        # (B, OC)
    out    = conv(x, wmod*demod, groups=batch)

Device decomposition per core (2 samples):
    conv(x, w*s) == conv(x*s, w)   -> fold s into the x fp32->bf16 pad/cast
    demod*SCALE  == rsqrt(ss + eps*IC*K*K) with ss = sum_ic WS[oc,ic]*s2[b,ic],
                    WS = sum_khkw w^2   -> one tiny PE matmul, applied as a
                    per-partition scalar on the PSUM->SBUF output copy.
    conv itself: 9 shifted bf16 matmuls x 4 ic-chunks accumulated in PSUM,
    N = 8 rows x 64 cols = 512 per matmul, x held SBUF-resident zero-padded
    to 66x66 per (sample, ic-chunk).
"""

import sys

for _p in ("/opt/trn_rl_repo",):
    if _p not in sys.path:
        sys.path.append(_p)

import numpy as np

import concourse.bass as bass
import concourse.tile as tile
from concourse import mybir
from concourse.bass_utils import run_bass_kernel_spmd

# ---------------------------------------------------------------------------
# Workaround for this container's walrus build: an instruction can carry only
# one semaphore wait (two for EventSemaphore), but Tile emits up to two per
# instruction (and the exit drain gets one per logical processor), which
# walrus rejects with "Too many sync wait commands". Fix at the BIR-JSON
# level: move excess waits onto NoOp carrier instructions inserted directly
# before the offender on the same engine — semantically identical (all waits
# still satisfied before the instruction executes, per-engine order kept).
# ---------------------------------------------------------------------------
import json as _json

_SPLIT_OK_ENGINES = {"PE", "DVE", "Activation", "Pool", "SP"}
_orig_to_json_bytes = bass.Bass.to_json_bytes


def _to_json_bytes_split_waits(self):
    raw = _orig_to_json_bytes(self)
    m = _json.loads(raw)
    changed = False
    for fn in m.get("functions", []):
        for bb in fn.get("blocks", []):
            insts = bb.get("instructions", [])
            new_insts = []
            for inst in insts:
                si = inst.get("sync_info")
                waits = (si or {}).get("on_wait") or []
                op = inst.get("opcode", "")
                limit = 2 if op == "EventSemaphore" else 1
                if len(waits) > limit:
                    eng = inst.get("engine")
                    assert eng in _SPLIT_OK_ENGINES, (
                        f"instruction {inst.get('name')} on engine {eng} has "
                        f"{len(waits)} waits; carrier NoOp not known-safe there"
                    )
                    changed = True
                    keep = waits[-limit:]
                    for i, w in enumerate(waits[:-limit]):
                        new_insts.append(
                            {
                                "debug": inst.get("debug", 0),
                                "engine": eng,
                                "ins": [],
                                "name": f"{inst['name']}.w{i}",
                                "opcode": "NoOp",
                                "outs": [],
                                "sync_info": {"on_wait": [w], "on_update": []},
                            }
                        )
                    si["on_wait"] = keep
                new_insts.append(inst)
            bb["instructions"] = new_insts
    if not changed:
        return raw
    return _json.dumps(m).encode()


bass.Bass.to_json_bytes = _to_json_bytes_split_waits

# ---------------------------------------------------------------------------
# Problem constants (hardcoded per spec)
# ---------------------------------------------------------------------------
B, IC, OC, H, W, KS, SD = 16, 512, 512, 64, 64, 3, 512
NCORES = 8
BPC = B // NCORES          # samples per core
P = 128
NIC = IC // P              # 4 ic chunks
NOC = OC // P              # 4 oc chunks
KK = KS * KS               # 9
PW = W + 2                 # 66 padded width
RB = 8                     # output rows per block
NBLK = H // RB             # 8 blocks
NFREE = RB * W             # 512 matmul free dim
# rsqrt(SCALE^2*ss + 1e-8) * SCALE == rsqrt(ss + 1e-8*IC*K*K)
EPS_FOLDED = 1e-8 * IC * KS * KS

F32 = mybir.dt.float32
BF16 = mybir.dt.bfloat16


def build_nc():
    nc = bass.Bass()
    xs = nc.dram_tensor("xs", [BPC, IC, H, W], F32, kind="ExternalInput")
    stT = nc.dram_tensor("stT", [SD, BPC], F32, kind="ExternalInput")
    wT = nc.dram_tensor("wT", [IC, KK, OC], F32, kind="ExternalInput")
    mwT = nc.dram_tensor("mwT", [SD, IC], F32, kind="ExternalInput")
    mb = nc.dram_tensor("mb", [IC], F32, kind="ExternalInput")
    out = nc.dram_tensor("out", [BPC, OC, H, W], F32, kind="ExternalOutput")

    with tile.TileContext(nc) as tc:
        with (
            tc.tile_pool(name="singles", bufs=1) as singles,
            tc.tile_pool(name="wstage", bufs=2) as wstage,
            tc.tile_pool(name="sqp", bufs=3) as sqp,
            tc.tile_pool(name="xstage", bufs=6) as xstage,
            tc.tile_pool(name="outp", bufs=6) as outp,
            tc.tile_pool(name="psum", bufs=8, space="PSUM") as psum,
        ):
            # ---- constants ------------------------------------------------
            mwT_sb = singles.tile([P, SD // P, IC], F32)
            mwT_v = mwT.rearrange("(ko ki) i -> ki ko i", ki=P)
            for k in range(SD // P):
                nc.sync.dma_start(mwT_sb[:, k], mwT_v[:, k])
            stT_sb = singles.tile([P, SD // P, BPC], F32)
            nc.sync.dma_start(stT_sb, stT.rearrange("(ko ki) b -> ki ko b", ki=P))
            mb_sb = singles.tile([P, NIC], F32)
            nc.sync.dma_start(mb_sb, mb.rearrange("(c p) -> p c", p=P))

            # ---- style projection: s[ic, b] = mod_w @ style.T + mod_b -----
            s_sb = singles.tile([P, NIC, BPC], F32)
            s2_sb = singles.tile([P, NIC, BPC], F32)
            for c in range(NIC):
                ps = psum.tile([P, NFREE], F32, tag="ps")
                for k in range(SD // P):
                    nc.tensor.matmul(
                        ps[:, :BPC],
                        mwT_sb[:, k, c * P : (c + 1) * P],
                        stT_sb[:, k, :],
                        start=(k == 0),
                        stop=(k == SD // P - 1),
                    )
                nc.vector.tensor_scalar_add(s_sb[:, c, :], ps[:, :BPC], mb_sb[:, c : c + 1])
                nc.vector.tensor_mul(s2_sb[:, c, :], s_sb[:, c, :], s_sb[:, c, :])

            # ---- x: pad to 66x66, scale by s[ic,b], cast to bf16 ----------
            # Prepared in 8-row bands (matching conv blocks, band-outer) so
            # conv block 0 can start as soon as its rows are staged; Tile's
            # AP-range overlap tracking gives per-band dependencies.
            xpad = singles.tile([P, BPC, NIC, PW * PW], BF16)

            def xprep(b):
                for c in range(NIC):
                    v = xpad[:, b, c, :].rearrange("p (r w) -> p r w", w=PW)
                    nc.gpsimd.memset(v[:, 0, :], 0.0)
                    nc.gpsimd.memset(v[:, PW - 1, :], 0.0)
                    nc.gpsimd.memset(v[:, 1 : PW - 1, 0:1], 0.0)
                    nc.gpsimd.memset(v[:, 1 : PW - 1, PW - 1 : PW], 0.0)
                for j in range(NBLK):
                    for c in range(NIC):
                        v = xpad[:, b, c, :].rearrange("p (r w) -> p r w", w=PW)
                        xst = xstage.tile([P, RB, W], F32, tag="xst")
                        nc.sync.dma_start(
                            xst, xs[b, c * P : (c + 1) * P, j * RB : (j + 1) * RB, :]
                        )
                        nc.scalar.activation(
                            out=v[:, 1 + j * RB : 1 + (j + 1) * RB, 1 : 1 + W],
                            in_=xst,
                            func=mybir.ActivationFunctionType.Copy,
                            scale=s_sb[:, c, b : b + 1],
                        )

            xprep(0)

            # ---- weights: load fp32, cast to bf16 -------------------------
            # cast on DVE (GpSimd measured ~3x slower). WS is computed later
            # from the bf16 copy (error on demod ~1e-4, immaterial), which
            # frees the fp32 staging slot right after the cast.
            wb = singles.tile([P, NIC, KK, OC], BF16)     # [ic, c, k, oc]
            ws = singles.tile([P, NIC, OC], F32)          # [ic, c, oc]
            for c in range(NIC):
                wst = wstage.tile([P, KK, OC], F32, tag="wst")
                nc.sync.dma_start(wst, wT[c * P : (c + 1) * P])
                nc.vector.tensor_copy(out=wb[:, c], in_=wst)          # cast

            # ---- WS[ic, oc] = sum_k wb^2 (squares ACT, add chain DVE) -----
            for c in range(NIC):
                nc.scalar.square(ws[:, c, :], wb[:, c, 0, :])
                for k in range(1, KK):
                    sq = sqp.tile([P, OC], F32, tag="sq")
                    nc.scalar.square(sq, wb[:, c, k, :])
                    nc.vector.tensor_add(ws[:, c, :], ws[:, c, :], sq)

            # ---- demod[oc, b] = 1/sqrt(WS.T @ s2 + eps') ------------------
            # Emitted lazily (after conv block 0) so the in-order PE stream
            # doesn't idle waiting on WS before starting conv; demod is only
            # needed by the first PSUM->SBUF output copy.
            dsq = singles.tile([P, NOC, BPC], F32)
            demod_sb = singles.tile([P, NOC, BPC], F32)
            eps_sb = singles.tile([P, 1], F32)
            nc.vector.memset(eps_sb, EPS_FOLDED)

            def demod_emit():
                for o in range(NOC):
                    pd = psum.tile([P, NFREE], F32, tag="ps")
                    for c in range(NIC):
                        nc.tensor.matmul(
                            pd[:, :BPC],
                            ws[:, c, o * P : (o + 1) * P],
                            s2_sb[:, c, :],
                            start=(c == 0),
                            stop=(c == NIC - 1),
                        )
                    nc.scalar.activation(
                        out=dsq[:, o, :],
                        in_=pd[:, :BPC],
                        func=mybir.ActivationFunctionType.Sqrt,
                        bias=eps_sb[:],
                        scale=1.0,
                    )
                    nc.vector.reciprocal(out=demod_sb[:, o, :], in_=dsq[:, o, :])

            # ---- conv: 9 shifted matmuls x 4 ic chunks --------------------
            # ic-chunk OUTER within each accumulation group so the first
            # matmuls only need chunk 0's weights/x while later chunks are
            # still in flight from HBM.
            def conv(b, jrange):
                for j in jrange:
                    for o in range(NOC):
                        ps = psum.tile([P, NFREE], F32, tag="ps")
                        idx = 0
                        for c in range(NIC):
                            for ky in range(KS):
                                for kx in range(KS):
                                    xv = xpad[:, b, c, :].rearrange(
                                        "p (r w) -> p r w", w=PW
                                    )[:, j * RB + ky : j * RB + ky + RB, kx : kx + W]
                                    nc.tensor.matmul(
                                        ps,
                                        wb[:, c, ky * KS + kx, o * P : (o + 1) * P],
                                        xv,
                                        start=(idx == 0),
                                        stop=(idx == KK * NIC - 1),
                                    )
                                    idx += 1
                        ot = outp.tile([P, RB, W], F32, tag="ot")
                        nc.vector.tensor_scalar_mul(
                            ot,
                            ps.rearrange("p (r w) -> p r w", w=W),
                            demod_sb[:, o, b : b + 1],
                        )
                        nc.sync.dma_start(
                            out[b, o * P : (o + 1) * P, j * RB : (j + 1) * RB, :], ot
                        )

            # Emit demod before conv so Tile sees the dataflow (writes precede
            # reads in trace order), but push its *scheduling* priority far
            # down so the in-order PE stream doesn't park on WS-gated demod
            # matmuls while conv work is ready. The scheduler slots demod in
            # when conv stalls on PSUM availability.
            with tc.high_priority(offset=-100000):
                demod_emit()
            conv(0, range(NBLK))
            xprep(1)
            conv(1, range(NBLK))

    return nc


_NC = None


def _get_nc():
    global _NC
    if _NC is None:
        _NC = build_nc()
    return _NC


def kernel(x, style, weight, mod_w, mod_b):
    x = np.ascontiguousarray(x, dtype=np.float32)
    style = np.asarray(style, dtype=np.float32)
    weight = np.asarray(weight, dtype=np.float32)
    mod_w = np.asarray(mod_w, dtype=np.float32)
    mod_b = np.ascontiguousarray(mod_b, dtype=np.float32)

    # host-side layout prep (replicated params)
    wT = np.ascontiguousarray(weight[0].transpose(1, 2, 3, 0)).reshape(IC, KK, OC)
    mwT = np.ascontiguousarray(mod_w.T)

    in_maps = []
    for i in range(NCORES):
        sl = slice(i * BPC, (i + 1) * BPC)
        in_maps.append(
            {
                "xs": np.ascontiguousarray(x[sl]),
                "stT": np.ascontiguousarray(style[sl].T),
                "wT": wT,
                "mwT": mwT,
                "mb": mod_b,
            }
        )

    nc = _get_nc()
    res = run_bass_kernel_spmd(nc, in_maps, core_ids=list(range(NCORES)))
    return np.concatenate([r["out"] for r in res.results], axis=0)



# revision 2
# speedup vs baseline: 1.0348x; 1.0348x over previous
"""Trainium2 Bass kernel: modulated (StyleGAN2) 3x3 conv, groups=batch,
via Winograd F(2x2, 3x3).

Full-input contract: kernel(**inputs) takes the unsharded numpy inputs and
returns the full (16, 512, 64, 64) fp32 output. Batch sharded 2-per-core
across 8 NeuronCores; weights replicated.

Host prep (fp32 numpy, exact):
    s      = style @ mod_w.T + mod_b                  # (B, IC)
    xpl    = bf16(x * s), padded-column-PARITY-SPLIT  # (B, IC, 64, 2, 34)
    Wt     = bf16(G w G^T)                            # (IC, 16, OC) Winograd wts
    demod  = rsqrt(s^2 @ WS.T + eps*IC*K*K)           # (B, OC), SCALE folded

The parity split (padded col pc = 2k -> plane 0 slot k, pc = 2k+1 ->
plane 1 slot k, planes padded to 34 for 4B alignment) makes every DVE
access pattern unit-stride, enabling the 2-elem/cycle 16-bit mode and
avoiding SBUF fetch waste. Same trick on the output: the device writes a
planar (ty, r, parity, tx) bf16 layout; the host interleaves + upcasts.

Device per core (2 samples; PE does ONLY the 16-position batched matmuls):
    per 256-tile block: input transform B^T d B as two add/sub stages
    (stage A rows, stage B cols), 256 matmuls (16 pos x 4 oc x 4 ic chunk,
    N=256 bf16), ACT drains PSUM with the demod scale fused, vertical +
    horizontal output transform A^T m A as adds, contiguous DMA out.
"""

import sys

for _p in ("/opt/trn_rl_repo",):
    if _p not in sys.path:
        sys.path.append(_p)

import numpy as np
import ml_dtypes

import concourse.bass as bass
import concourse.tile as tile
from concourse import mybir
from concourse.bass_utils import run_bass_kernel_spmd

# ---------------------------------------------------------------------------
# Walrus workaround (see baseline): split >1 semaphore waits per instruction
# onto NoOp carriers.
# ---------------------------------------------------------------------------
import json as _json

_SPLIT_OK_ENGINES = {"PE", "DVE", "Activation", "Pool", "SP"}
_orig_to_json_bytes = bass.Bass.to_json_bytes


def _to_json_bytes_split_waits(self):
    raw = _orig_to_json_bytes(self)
    m = _json.loads(raw)
    changed = False
    for fn in m.get("functions", []):
        for bb in fn.get("blocks", []):
            insts = bb.get("instructions", [])
            new_insts = []
            for inst in insts:
                si = inst.get("sync_info")
                waits = (si or {}).get("on_wait") or []
                op = inst.get("opcode", "")
                limit = 2 if op == "EventSemaphore" else 1
                if len(waits) > limit:
                    eng = inst.get("engine")
                    assert eng in _SPLIT_OK_ENGINES, (
                        f"instruction {inst.get('name')} on engine {eng} has "
                        f"{len(waits)} waits; carrier NoOp not known-safe there"
                    )
                    changed = True
                    keep = waits[-limit:]
                    for i, w in enumerate(waits[:-limit]):
                        new_insts.append(
                            {
                                "debug": inst.get("debug", 0),
                                "engine": eng,
                                "ins": [],
                                "name": f"{inst['name']}.w{i}",
                                "opcode": "NoOp",
                                "outs": [],
                                "sync_info": {"on_wait": [w], "on_update": []},
                            }
                        )
                    si["on_wait"] = keep
                new_insts.append(inst)
            bb["instructions"] = new_insts
    if not changed:
        return raw
    return _json.dumps(m).encode()


bass.Bass.to_json_bytes = _to_json_bytes_split_waits

# ---------------------------------------------------------------------------
# Problem constants (hardcoded per spec)
# ---------------------------------------------------------------------------
B, IC, OC, H, W, KS, SD = 16, 512, 512, 64, 64, 3, 512
NCORES = 8
BPC = B // NCORES           # samples per core
P = 128
NIC = IC // P               # 4 ic chunks
NOC = OC // P               # 4 oc chunks
EPS_FOLDED = 1e-8 * IC * KS * KS

TYB = 8                     # tile-rows per block
TB = TYB * 32               # tiles per block = 256 (matmul free dim)
NBLK = (H // 2) // TYB      # 4 blocks per sample
BROWS = 2 * TYB + 2         # 18 padded rows per band
PL = 34                     # parity-plane width (33 used + 1 alignment pad)

F32 = mybir.dt.float32
BF16 = mybir.dt.bfloat16
ADD = mybir.AluOpType.add
SUB = mybir.AluOpType.subtract

BF = ml_dtypes.bfloat16

# Winograd transform matrices (host side)
G_MAT = np.array([[1, 0, 0], [0.5, 0.5, 0.5], [0.5, -0.5, 0.5], [0, 0, 1]], np.float32)


def build_nc():
    nc = bass.Bass()
    # x: scaled bf16, padded row+col parity planes:
    # [b, ic, row-parity, 33 row slots, col-parity(2) * 34 col slots]
    xpl = nc.dram_tensor("xpl", [BPC, IC, 2, 33, 2 * PL], BF16, kind="ExternalInput")
    # weights partition-major: [ki, hmaj(=h*4+u), c, oc] so each per-h DMA
    # moves 16KB-contiguous runs per partition
    wt = nc.dram_tensor("wt", [P, 16, NIC, OC], BF16, kind="ExternalInput")
    dT = nc.dram_tensor("dT", [OC, BPC], F32, kind="ExternalInput")
    # out: planar bf16 [b, r, oc, parity, ty, tx]; host interleaves+upcasts
    opl = nc.dram_tensor("opl", [BPC, 2, OC, 2, 32, 32], BF16, kind="ExternalOutput")


    with tile.TileContext(nc) as tc:
        with (
            tc.tile_pool(name="singles", bufs=1) as singles,
            tc.tile_pool(name="bandp", bufs=2) as bandp,
            tc.tile_pool(name="vap", bufs=1) as vap,
            tc.tile_pool(name="xtp", bufs=9) as xtp,
            tc.tile_pool(name="mp", bufs=4) as mp,
            tc.tile_pool(name="pp", bufs=2) as ppool,
            tc.tile_pool(name="ysp", bufs=2) as ysp,
            tc.tile_pool(name="tmpp", bufs=4) as tmpp,
            tc.tile_pool(name="psum", bufs=4, space="PSUM") as psum,
        ):
            # ---- constants (weight DMAs emitted in the prologue below so
            # the first band's DMAs aren't queued behind them) ---------------
            d_sb = singles.tile([P, NOC, BPC], F32)
            wt_sb = singles.tile([P, 16, NIC, OC], BF16)

            blocks = [(s, blk) for s in range(BPC) for blk in range(NBLK)]

            # ---- band fill: DMA parity-split bf16 x rows -------------------
            band_tiles = {}

            def band_fill(bi):
                s, blk = blocks[bi]
                # [rp, 9 rows, col-planes]: even plane rows = padded rows
                # 16blk+2j (j 0..8), odd plane rows = 16blk+2j+1
                t = bandp.tile(
                    [P, NIC, 2, TYB + 1, 2, PL], BF16, tag="band", name=f"band{bi}"
                )
                band_tiles[bi] = t
                for rp in range(2):
                    for c in range(NIC):
                        nc.sync.dma_start(
                            t[:, c, rp],
                            xpl[
                                s, c * P : (c + 1) * P, rp,
                                TYB * blk : TYB * blk + TYB + 1,
                            ].rearrange("ki r (q l) -> ki r q l", l=PL),
                        )
                return t

            # ---- stage A: vertical input transform -------------------------
            # va[v][rows ty] over both parity planes at once (contiguous 68)
            va_tiles = {}

            def stage_a(bi):
                t = band_tiles[bi]
                va = vap.tile([P, NIC, 4, TYB, 2, PL], BF16, tag="va", name=f"va{bi}")
                va_tiles[bi] = va
                r0 = t[:, :, 0, 0:TYB]        # padded rows 16blk+ 0,2,..,14
                r1 = t[:, :, 1, 0:TYB]        # 1,3,..,15
                r2 = t[:, :, 0, 1 : TYB + 1]  # 2,4,..,16
                r3 = t[:, :, 1, 1 : TYB + 1]  # 3,5,..,17
                nc.vector.tensor_tensor(va[:, :, 0], r0, r2, SUB)
                nc.vector.tensor_tensor(va[:, :, 1], r1, r2, ADD)
                nc.vector.tensor_tensor(va[:, :, 2], r2, r1, SUB)
                nc.vector.tensor_tensor(va[:, :, 3], r1, r3, SUB)

            # ---- stage B: horizontal input transform (all unit-stride) -----
            # padded col pc=2k -> plane0[k], pc=2k+1 -> plane1[k]
            #   c0 (pc=2tx)   = plane0[0:32]   c2 (pc=2tx+2) = plane0[1:33]
            #   c1 (pc=2tx+1) = plane1[0:32]   c3 (pc=2tx+3) = plane1[1:33]
            xt_tiles = {}

            def stage_b(bi, h):
                va = va_tiles[bi]
                for v in range(4):
                    xt = xtp.tile([P, NIC, TB], BF16, tag="xt", name=f"xt{bi}_{v}_{h}")
                    xt_tiles[(bi, v, h)] = xt
                    o = xt.rearrange("p c (ty tx) -> p c ty tx", tx=32)
                    c0 = va[:, :, v, :, 0, 0:32]
                    c1 = va[:, :, v, :, 1, 0:32]
                    c2 = va[:, :, v, :, 0, 1:33]
                    c3 = va[:, :, v, :, 1, 1:33]
                    if h == 0:
                        nc.vector.tensor_tensor(o, c0, c2, SUB)
                    elif h == 1:
                        nc.vector.tensor_tensor(o, c1, c2, ADD)
                    elif h == 2:
                        nc.vector.tensor_tensor(o, c2, c1, SUB)
                    else:
                        nc.vector.tensor_tensor(o, c1, c3, SUB)

            # ---- deferred horizontal output transform + DMA out ------------
            pending = []

            def horizontal_flush():
                while pending:
                    pbi, pp, ys = pending.pop(0)
                    pb, pblk = blocks[pbi]
                    for r in range(2):
                        pr = pp[:, :, r]            # [P, NOC, 4, TB]
                        ye = ys[:, r, 0]            # [P, NOC, TB] contiguous
                        yo = ys[:, r, 1]
                        t3 = tmpp.tile([P, NOC, TB], BF16, tag="t", name=f"t3_{pbi}_{r}")
                        nc.vector.tensor_tensor(t3, pr[:, :, 0, :], pr[:, :, 1, :], ADD)
                        nc.vector.tensor_tensor(ye, t3, pr[:, :, 2, :], ADD)
                        t4 = tmpp.tile([P, NOC, TB], BF16, tag="t", name=f"t4_{pbi}_{r}")
                        nc.vector.tensor_tensor(t4, pr[:, :, 1, :], pr[:, :, 2, :], SUB)
                        nc.vector.tensor_tensor(yo, t4, pr[:, :, 3, :], SUB)
                    for o in range(NOC):
                        for r in range(2):
                            nc.sync.dma_start(
                                opl[
                                    pb, r, o * P : (o + 1) * P, :,
                                    pblk * 8 : (pblk + 1) * 8,
                                ],
                                ys[:, r, :, o].rearrange(
                                    "p q (ty tx) -> p q ty tx", tx=32
                                ),
                            )

            # ---- main loop -------------------------------------------------
            NB = len(blocks)
            band_fill(0)
            band_fill(1)
            nc.sync.dma_start(d_sb, dT.rearrange("(o ki) b -> ki o b", ki=P))
            # weight chunks in first-use order: 4KB/partition contiguous each
            for hm in range(16):
                nc.sync.dma_start(wt_sb[:, hm], wt[:, hm])
            stage_a(0)
            stage_b(0, 0)

            for bi in range(NB):
                s, blk = blocks[bi]
                b = s
                if bi + 2 < NB:
                    band_fill(bi + 2)

                pp_t = ppool.tile([P, NOC, 2, 4, TB], BF16, tag="pp", name=f"pp{bi}")
                # [r, parity, o, ty*tx]
                ys_t = ysp.tile([P, 2, 2, NOC, TB], BF16, tag="ys", name=f"ys{bi}")

                for h in range(4):
                    # DVE pipeline prefetches (before this h's drains)
                    if h < 3:
                        stage_b(bi, h + 1)
                    elif bi + 1 < NB:
                        stage_a(bi + 1)
                        stage_b(bi + 1, 0)
                    if h == 1:
                        horizontal_flush()

                    m_t = mp.tile([P, NOC, 4, TB], BF16, tag="m", name=f"m{bi}_{h}")

                    for o in range(NOC):
                        for up in range(2):
                            ps = psum.tile([P, 2, 512], F32, tag="ps", name=f"ps{bi}_{h}_{o}_{up}")
                            for ui in range(2):
                                u = 2 * up + ui
                                xt = xt_tiles[(bi, u, h)]
                                for c in range(NIC):
                                    nc.tensor.matmul(
                                        ps[:, ui, :TB],
                                        wt_sb[:, 4 * h + u, c, o * P : (o + 1) * P],
                                        xt[:, c, :],
                                        start=(c == 0),
                                        stop=(c == NIC - 1),
                                    )
                            # drain both u-banks with demod scale (ACT)
                            nc.scalar.activation(
                                out=m_t[:, o, 2 * up : 2 * up + 2, :],
                                in_=ps[:, :, :TB],
                                func=mybir.ActivationFunctionType.Copy,
                                scale=d_sb[:, o, b : b + 1],
                            )

                    # vertical output transform: P0 = m0+m1+m2, P1 = m1-m2-m3
                    e0 = nc.vector
                    e1 = nc.vector
                    t = tmpp.tile([P, NOC, TB], BF16, tag="t", name=f"tv{bi}_{h}")
                    e0.tensor_tensor(t, m_t[:, :, 0, :], m_t[:, :, 1, :], ADD)
                    e0.tensor_tensor(pp_t[:, :, 0, h, :], t, m_t[:, :, 2, :], ADD)
                    t2 = tmpp.tile([P, NOC, TB], BF16, tag="t", name=f"tv2{bi}_{h}")
                    e1.tensor_tensor(t2, m_t[:, :, 2, :], m_t[:, :, 3, :], ADD)
                    e1.tensor_tensor(pp_t[:, :, 1, h, :], m_t[:, :, 1, :], t2, SUB)

                pending.append((bi, pp_t, ys_t))

            horizontal_flush()

    return nc


_NC = None


def _get_nc():
    global _NC
    if _NC is None:
        _NC = build_nc()
    return _NC


def _host_prep(x, style, weight, mod_w, mod_b):
    x = np.asarray(x, np.float32)
    style = np.asarray(style, np.float32)
    w = np.asarray(weight, np.float32)[0]          # (OC, IC, 3, 3)
    mod_w = np.asarray(mod_w, np.float32)
    mod_b = np.asarray(mod_b, np.float32)

    s = style @ mod_w.T + mod_b                    # (B, IC)
    xs = (x * s[:, :, None, None]).astype(BF)      # (B, IC, H, W) bf16

    # padded row+col parity planes: padded row pr=2j -> row-plane0[j]
    # (= x row 2j-1), pr=2j+1 -> row-plane1[j] (= x row 2j); same for cols
    xpl = np.zeros((B, IC, 2, 33, 2, PL), dtype=BF)
    xpl[:, :, 0, 1:33, 0, 1:33] = xs[:, :, 1::2, 1::2]
    xpl[:, :, 0, 1:33, 1, 0:32] = xs[:, :, 1::2, 0::2]
    xpl[:, :, 1, 0:32, 0, 1:33] = xs[:, :, 0::2, 1::2]
    xpl[:, :, 1, 0:32, 1, 0:32] = xs[:, :, 0::2, 0::2]

    WS = (w * w).sum(axis=(2, 3))                  # (OC, IC)
    demod = 1.0 / np.sqrt((s * s) @ WS.T + EPS_FOLDED)   # (B, OC)

    Wt = np.einsum("uk,oikl,vl->oiuv", G_MAT, w, G_MAT)  # (OC, IC, 4, 4)
    # device layout [ki, hmaj=h*4+u, c, oc]: ic = c*128 + ki
    wt4 = Wt.reshape(OC, NIC, P, 4, 4)             # (oc, c, ki, u, h)
    wt = np.ascontiguousarray(
        wt4.transpose(2, 4, 3, 1, 0).reshape(P, 16, NIC, OC)
    ).astype(BF)                                   # (ki, h*4+u, c, oc)
    return xpl.reshape(B, IC, 2, 33, 2 * PL), wt, demod


def make_in_maps(inputs):
    xpl, wt, demod = _host_prep(**inputs)
    in_maps = []
    for i in range(NCORES):
        sl = slice(i * BPC, (i + 1) * BPC)
        in_maps.append(
            {
                "xpl": np.ascontiguousarray(xpl[sl]),
                "wt": wt,
                "dT": np.ascontiguousarray(demod[sl].T),
            }
        )
    return in_maps


def _post(res_list):
    # opl [BPC, r2, OC, p2, ty32, tx32] bf16 -> [BPC, OC, 64, 64] f32
    outs = []
    for r in res_list:
        a = np.asarray(r["opl"]).astype(np.float32)
        # -> [b, oc, ty, r, tx, p]
        a = a.transpose(0, 2, 4, 1, 5, 3).reshape(BPC, OC, H, W)
        outs.append(a)
    return np.concatenate(outs, axis=0)


def kernel(x, style, weight, mod_w, mod_b):
    in_maps = make_in_maps(
        dict(x=x, style=style, weight=weight, mod_w=mod_w, mod_b=mod_b)
    )
    nc = _get_nc()
    res = run_bass_kernel_spmd(nc, in_maps, core_ids=list(range(NCORES)))
    return _post(res.results)
